# revision 1
# baseline (speedup 1.0000x reference)
"""Trainium2 Bass kernel for nn_GroupProjection (gnn_message_passing).

Reference computation (B=8, N=8192, D=512, P=4, G=512, GS=16, 3 iters):
    for ite in range(3):
        delta = 0
        for i in range(P):
            gx = upd[:, groups[i], :]                 # gather
            dx = (1/(ite+1)) * gx @ W[i]              # GEMM
            delta[:, groups[i].ravel(), :] += dx      # scatter-add
        upd = upd + delta

Key identity: the gather index equals the scatter index, so
    delta[b, n, :] = (1/(ite+1)) * sum_i count_i[n] * (upd[b, n, :] @ W[i])
where count_i[n] = multiplicity of n in groups[i]. The gather/scatter
disappears into dense GEMMs plus a per-row weighted combine; counts are
computed on host with np.bincount (groups is only 32K ints).

Row n's update depends only on row n, so the whole 3-iteration computation
streams independently over 64 row-tiles of 128 per core, data-parallel over
the batch axis (1 batch element per NeuronCore).

Device layout per row-tile (n = 128 rows):
  updT  [d=4x128 part, n=128] bf16 (x arrives host-transposed+bf16-cast, so
        DMA loads it directly; bf16 keeps LDWEIGHTS overlapped with matmuls
        -- fp32/f32r matmuls are self-loading and serialize a ~200ns weight
        load into every matmul)
  per iter:  Y_i[n,128 x e,512] = sum_k matmul(lhsT=updT[k], rhs=W[i][k])
             delta[n,d] = sum_i (count_i*scale)[n] * Y_i
               (DVE handles Y0/Y1 via per-partition tensor_scalar ops, the
                idle Scalar engine pre-scales Y2/Y3, DVE adds them in)
             updT += transpose(delta)    (PE transpose, one fused DVE add)
  final out tile = transpose(updT) + delta -> DMA out (f32).
"""

import numpy as np

B, N, D = 8, 8192, 512
P = 4
NUM_ITER = 3
TP = 128                 # rows per tile
NT = N // TP             # 64 row tiles
KC = D // 128            # 4 contraction chunks
NCORES = 8

_CACHE = {}


def _build():
    import ml_dtypes
    import concourse.bass as bass
    import concourse.tile as tile
    from concourse import bacc, mybir

    f32 = mybir.dt.float32
    bf16 = mybir.dt.bfloat16
    MULT = mybir.AluOpType.mult
    ADD = mybir.AluOpType.add

    nc = bacc.Bacc("TRN2", target_bir_lowering=False, debug=False,
                   num_devices=NCORES)

    xT_d = nc.dram_tensor("xT", [D, N], bf16, kind="ExternalInput")
    w_d = nc.dram_tensor("w", [P, D, D], bf16, kind="ExternalInput")
    c_d = nc.dram_tensor("cnts", [128, NUM_ITER * NT * P], f32,
                         kind="ExternalInput")
    out_d = nc.dram_tensor("out", [N, D], f32, kind="ExternalOutput")
    ident_d = nc.inline_tensor(np.eye(128).astype(ml_dtypes.bfloat16), "ident")

    with tile.TileContext(nc) as tc:
        with (
            tc.tile_pool(name="const", bufs=1) as constp,
            tc.tile_pool(name="updT", bufs=4) as updTp,
            tc.tile_pool(name="delta", bufs=4) as deltap,
            tc.tile_pool(name="t23", bufs=4) as t23p,
            tc.tile_pool(name="outp", bufs=3) as outp,
            tc.tile_pool(name="psumY", bufs=7, space=bass.MemorySpace.PSUM) as psumY,
            tc.tile_pool(name="psumT", bufs=1, space=bass.MemorySpace.PSUM) as psumT,
        ):
            # W: chunk (i, k) lives at columns [(i*KC+k)*D : +D]
            w_sb = constp.tile([128, P * KC * D], bf16)
            for i in range(P):
                for k in range(KC):
                    nc.sync.dma_start(
                        w_sb[:, (i * KC + k) * D:(i * KC + k + 1) * D],
                        w_d[i, k * 128:(k + 1) * 128, :])
            cnt_sb = constp.tile([128, NUM_ITER * NT * P], f32)
            nc.sync.dma_start(cnt_sb[:], c_d[:])
            ident_sb = constp.tile([128, 128], bf16)
            nc.sync.dma_start(ident_sb[:], ident_d.ap())

            for t in range(NT):
                updT_t = updTp.tile([128, KC * 128], bf16, tag="updT")
                for k in range(KC):
                    nc.sync.dma_start(
                        updT_t[:, k * 128:(k + 1) * 128],
                        xT_d[k * 128:(k + 1) * 128, t * TP:(t + 1) * TP])
                for ite in range(NUM_ITER):
                    ys = []
                    for i in range(P):
                        y = psumY.tile([128, D], f32, tag="y")
                        for k in range(KC):
                            nc.tensor.matmul(
                                y[:],
                                updT_t[:, k * 128:(k + 1) * 128],
                                w_sb[:, (i * KC + k) * D:(i * KC + k + 1) * D],
                                start=(k == 0), stop=(k == KC - 1))
                        ys.append(y)
                    # delta = sum_i (cnt_i / (ite+1)) * Y_i (scale folded into
                    # cnts). ACT pre-scales Y2/Y3; DVE does Y0/Y1 + the adds.
                    cb = (ite * NT + t) * P
                    t0 = t23p.tile([128, D], bf16, tag="t23")
                    nc.scalar.mul(t0[:], ys[0][:], cnt_sb[:, cb:cb + 1])
                    t1 = t23p.tile([128, D], bf16, tag="t23")
                    nc.scalar.mul(t1[:], ys[1][:], cnt_sb[:, cb + 1:cb + 2])
                    delta_t = deltap.tile([128, D], bf16, tag="delta")
                    nc.vector.tensor_scalar_mul(delta_t[:], ys[2][:],
                                                cnt_sb[:, cb + 2:cb + 3])
                    nc.vector.scalar_tensor_tensor(
                        delta_t[:], ys[3][:], cnt_sb[:, cb + 3:cb + 4],
                        delta_t[:], MULT, ADD)
                    nc.vector.tensor_add(delta_t[:], delta_t[:], t0[:])
                    nc.vector.tensor_add(delta_t[:], delta_t[:], t1[:])
                    tp = psumT.tile([128, KC * 128], bf16, tag="tp")
                    if ite < NUM_ITER - 1:
                        new_updT = updTp.tile([128, KC * 128], bf16, tag="updT")
                        for k in range(KC):
                            nc.tensor.transpose(
                                tp[:, k * 128:(k + 1) * 128],
                                delta_t[:, k * 128:(k + 1) * 128], ident_sb[:])
                        nc.vector.tensor_add(new_updT[:], updT_t[:], tp[:])
                        updT_t = new_updT
                    else:
                        out_t = outp.tile([128, D], f32, tag="out")
                        for k in range(KC):
                            nc.tensor.transpose(
                                tp[:, k * 128:(k + 1) * 128],
                                updT_t[:, k * 128:(k + 1) * 128], ident_sb[:])
                        nc.vector.tensor_add(out_t[:], tp[:], delta_t[:])
                        nc.sync.dma_start(out_d[t * TP:(t + 1) * TP, :], out_t[:])
    nc.compile()
    return nc


def _prep_inputs(x, W, groups):
    import ml_dtypes

    bf16 = ml_dtypes.bfloat16
    cnt = np.stack([np.bincount(groups[i].ravel().astype(np.int64), minlength=N)
                    for i in range(P)]).astype(np.float32)        # [P, N]
    # cnts_host[p, (ite*NT + t)*P + i] = cnt[i, t*128+p] / (ite+1)
    cnts = np.empty((128, NUM_ITER * NT * P), dtype=np.float32)
    c_tiles = cnt.reshape(P, NT, TP)                              # [P, NT, 128]
    for ite in range(NUM_ITER):
        blk = (c_tiles / (ite + 1)).transpose(2, 1, 0)            # [128, NT, P]
        cnts[:, ite * NT * P:(ite + 1) * NT * P] = blk.reshape(128, NT * P)
    Wb = np.ascontiguousarray(W.astype(bf16))
    in_maps = []
    for b in range(B):
        xT = np.ascontiguousarray(x[b].T.astype(bf16))
        in_maps.append({"xT": xT, "w": Wb, "cnts": cnts})
    return in_maps


def kernel(x, W, groups, _trace=False, _trace_kwargs=None):
    from concourse.bass_utils import run_bass_kernel_spmd

    if "nc" not in _CACHE:
        _CACHE["nc"] = _build()
    nc = _CACHE["nc"]

    in_maps = _prep_inputs(np.asarray(x), np.asarray(W), np.asarray(groups))
    kw = {}
    if _trace:
        kw = {"trace": True, **(_trace_kwargs or {})}
    res = run_bass_kernel_spmd(nc, in_maps, core_ids=list(range(NCORES)), **kw)
    _CACHE["last_result"] = res
    out = np.stack([res.results[b]["out"] for b in range(B)]).astype(np.float32)
    return out



# revision 22
# speedup vs baseline: 1.5681x; 1.5681x over previous
"""Trainium2 Bass kernel for nn_GroupProjection (gnn_message_passing).

Reference computation (B=8, N=8192, D=512, P=4, G=512, GS=16, 3 iters):
    for ite in range(3):
        delta = 0
        for i in range(P):
            gx = upd[:, groups[i], :]                 # gather
            dx = (1/(ite+1)) * gx @ W[i]              # GEMM
            delta[:, groups[i].ravel(), :] += dx      # scatter-add
        upd = upd + delta

Key identity: gather index == scatter index, so
    delta[b, n, :] = (1/(ite+1)) * sum_i cnt_i[n] * (upd[b, n, :] @ W[i])
with cnt_i[n] = multiplicity of n in groups[i] (host-side bincount).

This version keeps the whole recurrence in the TRANSPOSED layout
updT[d, n] so no PE transposes are ever needed:

  V_i[d, n] = cnt_i[n] * updT[d, n]          (DVE mult, counts broadcast
                                              across partitions, iteration
                                              scale folded into W)
  deltaT[e-chunk, n] = sum_{i,dk} W[i][dk,e]^T @ V_i[dk, n]
                                             (stationary = W chunk [128x128],
                                              moving = V [128 x 512], all 16
                                              products PSUM-accumulated)
  updT += deltaT                             (DVE add, PSUM + SBUF -> SBUF)

The final iteration instead emits the row-major result directly:
  out[n, e] = sum_{i,dk} V_i[dk, n-slice]^T @ (W[i][dk, :]/3)
            + sum_dk upd[dk, n-slice]^T @ I_dk     ("identity projection"
                                                    folds the +upd)
so the output DMA is contiguous. Work is tiled over 16 super-tiles of 512
particles; batch is data-parallel, 1 element per NeuronCore.
"""

import os

import numpy as np

_REPS = int(os.environ.get("BENCH_REPS", "1"))  # timing amplification only

B, N, D = 8, 8192, 512
P = 4
NUM_ITER = 3
TS = 512                 # rows per super-tile
NTS = N // TS            # 16 super-tiles
KC = D // 128            # 4 contraction chunks of 128
NCORES = 8

_CACHE = {}


def _build():
    import concourse.bass as bass
    import concourse.tile as tile
    from concourse import bacc, mybir

    f32 = mybir.dt.float32
    bf16 = mybir.dt.bfloat16

    nc = bacc.Bacc("TRN2", target_bir_lowering=False, debug=False,
                   num_devices=NCORES)

    xT_d = nc.dram_tensor("xT", [D, N], bf16, kind="ExternalInput")
    wB_d = nc.dram_tensor("wB", [128, 2 * P * KC * KC * 128], bf16,
                          kind="ExternalInput")
    wA_d = nc.dram_tensor("wA", [128, P * KC * D], bf16, kind="ExternalInput")
    id_d = nc.dram_tensor("idt", [128, 128], bf16, kind="ExternalInput")
    cb_d = nc.dram_tensor("cntb", [128, NTS * P * TS], bf16,
                          kind="ExternalInput")
    out_d = nc.dram_tensor("out", [N, D], f32, kind="ExternalOutput")

    # (dk, i) issue order: chunks with dk=3 are consumed last, giving the
    # DVE time to finish the adds/mults that produce them.
    order = [(dk, i) for dk in range(KC) for i in range(P)]

    with tile.TileContext(nc) as tc:
        with (
            tc.tile_pool(name="const", bufs=1) as constp,
            tc.tile_pool(name="upd", bufs=6) as up,
            tc.tile_pool(name="v", bufs=3) as vp,
            tc.tile_pool(name="cnt", bufs=3) as cntp,
            tc.tile_pool(name="outp", bufs=4) as outp,
            tc.tile_pool(name="psum", bufs=8, space=bass.MemorySpace.PSUM) as psump,
        ):
            # Constants are DMA'd after super-tile 0's inputs: the SP HWDGE
            # ring is FIFO, so this lets the first V-mults/matmuls start as
            # soon as the wB piece they need lands instead of behind ~7MB.
            wB = constp.tile([128, 2 * P * KC * KC * 128], bf16)
            wA = constp.tile([128, P * KC * D], bf16)
            idt = constp.tile([128, 128], bf16)
            WCHUNK = 2 * P * KC * KC * 128 // 8    # one (ite, ec) group

            nc.sync.dma_start(wB[:, 0:WCHUNK], wB_d[:, 0:WCHUNK])
            first = True
            for st in [s for _ in range(_REPS) for s in range(NTS)]:
                # u chunk 0 then counts first: the dk-major V-mult order
                # needs exactly these two to start.
                u = up.tile([128, KC * TS], bf16, tag="u")
                nc.sync.dma_start(u[:, 0:TS], xT_d[0:128, st * TS:(st + 1) * TS])
                c = cntp.tile([128, P * TS], bf16, tag="c")
                nc.sync.dma_start(
                    c[:], cb_d[:, st * P * TS:(st + 1) * P * TS])
                for dk in range(1, KC):
                    nc.sync.dma_start(
                        u[:, dk * TS:(dk + 1) * TS],
                        xT_d[dk * 128:(dk + 1) * 128, st * TS:(st + 1) * TS])
                if first:
                    first = False
                    for w0 in range(1, 4):
                        nc.sync.dma_start(
                            wB[:, w0 * WCHUNK:(w0 + 1) * WCHUNK],
                            wB_d[:, w0 * WCHUNK:(w0 + 1) * WCHUNK])
                    nc.sync.dma_start(idt[:], id_d[:])
                    nc.sync.dma_start(wA[:], wA_d[:])
                    for w0 in range(4, 8):
                        nc.sync.dma_start(
                            wB[:, w0 * WCHUNK:(w0 + 1) * WCHUNK],
                            wB_d[:, w0 * WCHUNK:(w0 + 1) * WCHUNK])

                for ite in range(NUM_ITER):
                    v = vp.tile([128, P * KC * TS], bf16, tag="v")
                    for dk, i in order:
                        nc.vector.tensor_mul(
                            v[:, (i * KC + dk) * TS:(i * KC + dk + 1) * TS],
                            u[:, dk * TS:(dk + 1) * TS],
                            c[:, i * TS:(i + 1) * TS])
                    if ite < NUM_ITER - 1:
                        u_new = up.tile([128, KC * TS], bf16, tag="u")
                        for ec in range(KC):
                            y = psump.tile([128, TS], f32, tag="y")
                            for idx, (dk, i) in enumerate(order):
                                cB = ((ite * KC + ec) * KC + dk) * P + i
                                nc.tensor.matmul(
                                    y[:],
                                    wB[:, cB * 128:(cB + 1) * 128],
                                    v[:, (i * KC + dk) * TS:(i * KC + dk + 1) * TS],
                                    start=(idx == 0), stop=(idx == 15))
                            nc.vector.tensor_add(
                                u_new[:, ec * TS:(ec + 1) * TS], y[:],
                                u[:, ec * TS:(ec + 1) * TS])
                        u = u_new
                    else:
                        for t in range(KC):
                            o = psump.tile([128, D], f32, tag="y")
                            # identity-projection (+upd): rhs is zero outside
                            # a 128-wide block, so write only that PSUM slice.
                            # start=True on the first clears has_written for
                            # the whole bank; the 4 slices cover all 512 cols.
                            for dk in range(KC):
                                nc.tensor.matmul(
                                    o[:, dk * 128:(dk + 1) * 128],
                                    u[:, dk * TS + t * 128:dk * TS + t * 128 + 128],
                                    idt[:],
                                    start=(dk == 0), stop=False)
                            for idx, (dk, i) in enumerate(order):
                                base = (i * KC + dk) * TS + t * 128
                                nc.tensor.matmul(
                                    o[:],
                                    v[:, base:base + 128],
                                    wA[:, (i * KC + dk) * D:(i * KC + dk + 1) * D],
                                    start=False, stop=(idx == 15))
                            ob = outp.tile([128, D], f32, tag="out")
                            nc.scalar.copy(ob[:], o[:])
                            # Output DMAs ride the Activation HWDGE ring so
                            # they never queue behind input prefetches (SP).
                            nc.scalar.dma_start(
                                out_d[st * TS + t * 128:st * TS + (t + 1) * 128, :],
                                ob[:])
    nc.compile()
    return nc


def _prep_inputs(x, W, groups):
    import ml_dtypes

    bf16 = ml_dtypes.bfloat16
    cnt = np.stack([np.bincount(groups[i].ravel().astype(np.int64), minlength=N)
                    for i in range(P)]).astype(np.float32)        # [P, N]

    W4 = W.astype(np.float32).reshape(P, KC, 128, KC, 128)
    # wB[p, (((ite*KC + ec)*KC + dk)*P + i)*128 + col] = W[i][dk*128+p, ec*128+col]/(ite+1)
    arr = W4.transpose(2, 3, 1, 0, 4).reshape(128, P * KC * KC * 128)
    wB = np.concatenate([arr, arr / 2], axis=1).astype(bf16)
    # wA[p, (i*KC+dk)*512 + e] = W[i][dk*128+p, e]/3
    wA = (W.astype(np.float32).reshape(P, KC, 128, D)
          .transpose(2, 0, 1, 3).reshape(128, P * KC * D) / 3).astype(bf16)
    idt = np.eye(128, dtype=np.float32).astype(bf16)
    # cntb[p, ((st*P)+i)*TS + nn] = cnt[i][st*TS+nn]
    cntb = np.ascontiguousarray(
        np.broadcast_to(
            cnt.reshape(P, NTS, TS).transpose(1, 0, 2).reshape(1, NTS * P * TS),
            (128, NTS * P * TS))).astype(bf16)

    in_maps = []
    for b in range(B):
        xT = np.ascontiguousarray(x[b].T.astype(bf16))
        in_maps.append({"xT": xT, "wB": wB, "wA": wA, "idt": idt, "cntb": cntb})
    return in_maps


def kernel(x, W, groups, _trace=False, _trace_kwargs=None):
    from concourse.bass_utils import run_bass_kernel_spmd

    if "nc" not in _CACHE:
        _CACHE["nc"] = _build()
    nc = _CACHE["nc"]

    in_maps = _prep_inputs(np.asarray(x), np.asarray(W), np.asarray(groups))
    kw = {}
    if _trace:
        kw = {"trace": True, **(_trace_kwargs or {})}
    res = run_bass_kernel_spmd(nc, in_maps, core_ids=list(range(NCORES)), **kw)
    _CACHE["last_result"] = res
    out = np.stack([res.results[b]["out"] for b in range(B)]).astype(np.float32)
    return out
